# revision 1
# baseline (speedup 1.0000x reference)
"""CrystalGraphConv Bass kernel for 8 TRN2 NeuronCores.

Strategy (edge-parallel, dst-sharded):
  - Nodes partitioned into 8 contiguous ranges of 1250. Edge e is owned by the
    core owning dst[e], so segment_sum is core-local (no big all-reduce).
  - Per core, dst-space is split into 10 windows of 128 nodes. Edges grouped by
    window; per-window tile counts are the max over cores so the SPMD program is
    identical on every core (pad edges contribute zero via an out-of-range
    one-hot column).
  - src/dst features gathered edge-major via indirect DMA, transposed on the
    TensorEngine to feature-major for the edge-MLP matmuls; the edge-major src
    tiles feed the gated-message multiply directly.
  - Scatter = matmul(lhsT=msg[e,f], rhs=onehot[e,d]) accumulated into a
    [128, 1280] f32 PSUM region (one-hot built on DVE via is_equal vs iota).
  - Node MLP + BN are node-sharded; BN statistics via a tiny [128,2] AllReduce.
"""

import sys, time

sys.path.insert(0, "/opt/trn_rl_repo")

import numpy as np
import ml_dtypes

import concourse.bacc as bacc
import concourse.bass as bass
import concourse.mybir as mybir
import concourse.tile as tile
from concourse import library_config
from concourse.bass_utils import run_bass_kernel_spmd
from concourse.masks import make_identity

import os

BF16 = ml_dtypes.bfloat16
USE_COLL = os.environ.get("K_USE_COLL", "1") == "1"
SKIP_GATHER = os.environ.get("K_SKIP_GATHER", "0") == "1"
N_CORES = 8
P = 128
WIN = 128  # dst window width (nodes per scatter window)
BN_EPS = 1e-5
PAD_OFF = 200.0  # dst_off for pad edges; >= WIN so one-hot row is all zeros
F32 = mybir.dt.float32
BT = mybir.dt.bfloat16
I16 = mybir.dt.int16
AF = mybir.ActivationFunctionType
OP = mybir.AluOpType


def _wrap_idx(flat: np.ndarray) -> np.ndarray:
    """dma_gather index layout: flat i lives at partition i%16, col i//16,
    replicated across the 8 partition groups (rows 16..127 mirror 0..15)."""
    assert flat.size % 16 == 0
    a = flat.reshape(-1, 16).T.astype(np.int16)  # [16, n/16]
    return np.tile(a, (8, 1))  # [128, n/16]


def _prep(node_features, edge_features, edge_index):
    """Host-side sharding/schedule. Returns (schedule, per-core input dicts)."""
    N, H = node_features.shape
    E = edge_index.shape[1]
    ED = edge_features.shape[1]
    n_local = (N + N_CORES - 1) // N_CORES  # 1250
    n_win = (n_local + WIN - 1) // WIN  # 10
    n_loc_pad = n_win * WIN  # 1280

    src = edge_index[0].astype(np.int64)
    dst = edge_index[1].astype(np.int64)
    core_of = np.minimum(dst // n_local, N_CORES - 1)

    # per (core, window) edge-id lists
    per_core = []
    counts = np.zeros((N_CORES, n_win), dtype=np.int64)
    for c in range(N_CORES):
        eids = np.nonzero(core_of == c)[0]
        loc = dst[eids] - c * n_local
        w = loc >> 7
        order = np.argsort(w, kind="stable")
        eids = eids[order]
        w = w[order]
        counts[c] = np.bincount(w, minlength=n_win)
        per_core.append(eids)

    tiles_w = np.maximum(1, (counts.max(axis=0) + P - 1) // P).astype(np.int64)
    E_w = tiles_w * P  # padded edges per window (same all cores)
    O_w = np.concatenate([[0], np.cumsum(E_w)])  # window offsets
    E_CAP = int(O_w[-1])
    T_w = np.concatenate([[0], np.cumsum(tiles_w)])  # tile offsets
    T_tot = int(T_w[-1])

    nf32 = np.asarray(node_features, dtype=np.float32)
    nf_pad = np.zeros((N_CORES * n_loc_pad if N_CORES * n_loc_pad > N else N, H),
                      dtype=np.float32)
    nf_pad[:N] = nf32
    ef32 = np.asarray(edge_features, dtype=np.float32)

    in_maps = []
    for c in range(N_CORES):
        eids = per_core[c]
        loc_all = dst[eids] - c * n_local
        w_all = loc_all >> 7
        # build padded flat edge list
        g_src = np.zeros(E_CAP, dtype=np.int64)
        g_dst = np.zeros(E_CAP, dtype=np.int64)
        efT = np.zeros((64, E_CAP), dtype=BF16)
        doff = np.full((P, T_tot), PAD_OFF, dtype=np.float32)
        pos = 0
        for w in range(n_win):
            ids = eids[w_all == w]
            k = len(ids)
            o = int(O_w[w])
            g_src[o:o + k] = src[ids]
            g_dst[o:o + k] = dst[ids]
            efT[:, o:o + k] = ef32[ids].T.astype(BF16)
            offs = (dst[ids] - c * n_local - w * WIN).astype(np.float32)
            t0 = int(T_w[w])
            full = np.full(int(E_w[w]), PAD_OFF, dtype=np.float32)
            full[:k] = offs
            doff[:, t0:t0 + int(tiles_w[w])] = full.reshape(-1, P).T
        # per-tile index columns for indirect DMA: [128, T_tot] int32
        sidx32 = g_src.reshape(-1, P).T.astype(np.int32)
        didx32 = g_dst.reshape(-1, P).T.astype(np.int32)

        lo = c * n_local
        nfT_slice = nf_pad[lo:lo + n_loc_pad].T.astype(BF16).copy()  # [128,1280]
        nf32_slice = nf_pad[lo:lo + n_loc_pad].copy()  # [1280,128] f32

        in_maps.append({
            "nf_tab": nf32[:N].astype(BF16),
            "efT": efT,
            "sidx": sidx32,
            "didx": didx32,
            "doff": doff.astype(BF16),
            "nfT": nfT_slice,
            "nf32": nf32_slice,
        })

    sched = dict(N=N, H=H, ED=ED, n_local=n_local, n_win=n_win,
                 n_loc_pad=n_loc_pad, E_CAP=E_CAP,
                 tiles_w=tiles_w.tolist(), E_w=E_w.tolist(),
                 O_w=O_w.tolist(), T_w=T_w.tolist(), T_tot=T_tot)
    return sched, in_maps


def _shared_inputs(We1, be1, We2, be2, Wn1, bn1, Wn2, bn2, gamma, beta, H):
    col = lambda v: np.asarray(v, np.float32).reshape(H, 1)
    return {
        "w_src": np.asarray(We1[:H], BF16),
        "w_dst": np.asarray(We1[H:2 * H], BF16),
        "w_ef": np.asarray(We1[2 * H:], BF16),
        "we2": np.asarray(We2, BF16),
        "wn1a": np.asarray(Wn1[:H], BF16),
        "wn1b": np.asarray(Wn1[H:], BF16),
        "wn2": np.asarray(Wn2, BF16),
        "be1": col(be1),
        "be2b": np.tile(np.asarray(be2, np.float32)[None, :], (P, 1)),
        "bn1": col(bn1),
        "bn2": col(bn2),
        "gam": col(gamma),
        "bet": col(beta),
        "iota": np.tile(np.arange(WIN, dtype=np.float32)[None, :],
                        (P, 1)).astype(BF16),
    }


def _build_program(s):
    H = s["H"]
    n_win, n_loc_pad = s["n_win"], s["n_loc_pad"]
    E_CAP, T_tot = s["E_CAP"], s["T_tot"]
    tiles_w, E_w, O_w, T_w = s["tiles_w"], s["E_w"], s["O_w"], s["T_w"]
    E_w_max = max(E_w)
    N_REAL = s["n_local"]  # real nodes per core

    nc = bacc.Bacc("TRN2", target_bir_lowering=False, debug=False,
                   num_devices=N_CORES)
    dt = lambda n, sh, d, k: nc.dram_tensor(n, sh, d, kind=k).ap()
    IN = "ExternalInput"
    nf_tab = dt("nf_tab", [s["N"], H], BT, IN)
    efT_d = dt("efT", [64, E_CAP], BT, IN)
    sidx_d = dt("sidx", [P, T_tot], mybir.dt.int32, IN)
    didx_d = dt("didx", [P, T_tot], mybir.dt.int32, IN)
    doff_d = dt("doff", [P, T_tot], BT, IN)
    nfT_d = dt("nfT", [P, n_loc_pad], BT, IN)
    nf32_d = dt("nf32", [n_loc_pad, H], F32, IN)
    wname = ["w_src", "w_dst", "w_ef", "we2", "wn1a", "wn1b", "wn2"]
    wshape = {"w_ef": [64, H]}
    wd = {n: dt(n, wshape.get(n, [H, H]), BT, IN) for n in wname}
    bname = ["be1", "bn1", "bn2", "gam", "bet"]
    bd = {n: dt(n, [P, 1], F32, IN) for n in bname}
    be2b_d = dt("be2b", [P, H], F32, IN)
    iota_d = dt("iota", [P, WIN], BT, IN)
    out_d = dt("out", [n_loc_pad, H], F32, "ExternalOutput")

    with tile.TileContext(nc) as tc:
        with tc.tile_pool(name="const", bufs=1) as cp, \
             tc.tile_pool(name="aggps", bufs=1, space="PSUM") as aggpool:
            # ---- persistent constants to SBUF ----
            ws = {}
            for n in wname:
                t = cp.tile(wshape.get(n, [H, H]), BT, tag=f"w_{n}")
                nc.sync.dma_start(t[:], wd[n][:])
                ws[n] = t
            bs = {}
            for n in bname:
                t = cp.tile([P, 1], F32, tag=f"b_{n}")
                nc.sync.dma_start(t[:], bd[n][:])
                bs[n] = t
            be2b = cp.tile([P, H], F32, tag="be2b")
            nc.sync.dma_start(be2b[:], be2b_d[:])
            iota = cp.tile([P, WIN], BT, tag="iota")
            nc.sync.dma_start(iota[:], iota_d[:])
            sidx = cp.tile([P, T_tot], mybir.dt.int32, tag="sidx")
            nc.sync.dma_start(sidx[:], sidx_d[:])
            didx = cp.tile([P, T_tot], mybir.dt.int32, tag="didx")
            nc.sync.dma_start(didx[:], didx_d[:])
            doff = cp.tile([P, T_tot], BT, tag="doff")
            nc.sync.dma_start(doff[:], doff_d[:])
            zlhs = cp.tile([P, P], BT, tag="zlhs")
            nc.vector.memset(zlhs[:], 0.0)
            zrhs = cp.tile([P, 512], BT, tag="zrhs")
            nc.vector.memset(zrhs[:], 0.0)
            identE = cp.tile([P, P], BT, tag="identE")
            make_identity(nc, identE[:])

            agg = aggpool.tile([P, n_loc_pad], F32, tag="agg")
            # zero-init agg (sets has_written so scatter mms accumulate)
            for a in range(0, n_loc_pad, 512):
                n = min(512, n_loc_pad - a)
                nc.tensor.matmul(agg[:, a:a + n], zlhs[:], zrhs[:, :n],
                                 start=True, stop=True)

            # ---- edge phase ----
            with tc.tile_pool(name="gath", bufs=2) as gp, \
                 tc.tile_pool(name="work", bufs=2) as wp, \
                 tc.tile_pool(name="small", bufs=3) as sp, \
                 tc.tile_pool(name="hps", bufs=2, space="PSUM") as hpp, \
                 tc.tile_pool(name="wps", bufs=2, space="PSUM") as wpp:
                for w in range(n_win):
                    ew, tw, o, t0 = E_w[w], tiles_w[w], O_w[w], T_w[w]
                    sEM = gp.tile([P, E_w_max], BT, tag="sEM")
                    srcT_b = gp.tile([P, E_w_max], BT, tag="srcT")
                    dstT_b = gp.tile([P, E_w_max], BT, tag="dstT")
                    for t in range(tw):
                        cs = slice(t * P, (t + 1) * P)
                        nc.gpsimd.indirect_dma_start(
                            sEM[:, cs], None, nf_tab[:],
                            bass.IndirectOffsetOnAxis(ap=sidx[:, t0 + t:t0 + t + 1], axis=0))
                        dEM = sp.tile([P, P], BT, tag="dEM")
                        nc.gpsimd.indirect_dma_start(
                            dEM[:], None, nf_tab[:],
                            bass.IndirectOffsetOnAxis(ap=didx[:, t0 + t:t0 + t + 1], axis=0))
                        tp_s = wpp.tile([P, P], BT, tag="wps")
                        nc.tensor.transpose(tp_s[:], sEM[:, cs], identE[:])
                        nc.vector.tensor_copy(srcT_b[:, cs], tp_s[:])
                        tp_d = wpp.tile([P, P], BT, tag="wps")
                        nc.tensor.transpose(tp_d[:], dEM[:], identE[:])
                        nc.vector.tensor_copy(dstT_b[:, cs], tp_d[:])
                    efw = gp.tile([64, E_w_max], BT, tag="efw")
                    nc.sync.dma_start(efw[:, :ew], efT_d[:, o:o + ew])

                    srcT = srcT_b[:, 0:ew]
                    dstT = dstT_b[:, 0:ew]

                    hsb = wp.tile([P, E_w_max], BT, tag="hsb")
                    for a in range(0, ew, 512):
                        n = min(512, ew - a)
                        hp = hpp.tile([P, 512], F32, tag="hp")
                        nc.tensor.matmul(hp[:, :n], ws["w_src"][:], srcT[:, a:a + n],
                                         start=True, stop=False)
                        nc.tensor.matmul(hp[:, :n], ws["w_dst"][:], dstT[:, a:a + n],
                                         start=False, stop=False)
                        nc.tensor.matmul(hp[:, :n], ws["w_ef"][:], efw[:, a:a + n],
                                         start=False, stop=True)
                        nc.vector.tensor_scalar(hsb[:, a:a + n], hp[:, :n],
                                                bs["be1"][:], 0.0,
                                                op0=OP.add, op1=OP.max)
                    wb = wp.tile([P, E_w_max], BT, tag="wb")
                    for t in range(tw):
                        wps_t = wpp.tile([P, P], F32, tag="wps")
                        nc.tensor.matmul(wps_t[:], hsb[:, t * P:(t + 1) * P],
                                         ws["we2"][:], start=True, stop=True)
                        nc.vector.tensor_tensor(wb[:, t * P:(t + 1) * P],
                                                wps_t[:], be2b[:], op=OP.add)
                    sg = wp.tile([P, E_w_max], BT, tag="sg")
                    nc.scalar.activation(sg[:, :ew], wb[:, :ew], AF.Sigmoid)
                    for t in range(tw):
                        msg = sp.tile([P, P], BT, tag="msg")
                        nc.vector.tensor_tensor(msg[:], sEM[:, t * P:(t + 1) * P],
                                                sg[:, t * P:(t + 1) * P], op=OP.mult)
                        hot = sp.tile([P, WIN], BT, tag="hot")
                        nc.vector.tensor_tensor(
                            hot[:], doff[:, t0 + t:t0 + t + 1].to_broadcast([P, WIN]),
                            iota[:], op=OP.is_equal)
                        nc.tensor.matmul(agg[:, w * WIN:(w + 1) * WIN],
                                         msg[:], hot[:], start=False, stop=True)

            # ---- node phase ----
            with tc.tile_pool(name="node", bufs=1) as np_, \
                 tc.tile_pool(name="nps", bufs=2, space="PSUM") as npp, \
                 tc.tile_pool(name="tps", bufs=2, space="PSUM") as tpp, \
                 tc.tile_pool(name="ntmp", bufs=2) as nt, \
                 tc.tile_pool(name="dram", bufs=1, space="DRAM") as dp:
                aggsb = np_.tile([P, n_loc_pad], BT, tag="aggsb")
                nc.vector.tensor_copy(aggsb[:], agg[:])
                nfT = np_.tile([P, n_loc_pad], BT, tag="nfT")
                nc.sync.dma_start(nfT[:], nfT_d[:])
                u1 = np_.tile([P, n_loc_pad], BT, tag="u1")
                for a in range(0, n_loc_pad, 512):
                    n = min(512, n_loc_pad - a)
                    up = npp.tile([P, 512], F32, tag="up")
                    nc.tensor.matmul(up[:, :n], ws["wn1a"][:], nfT[:, a:a + n],
                                     start=True, stop=False)
                    nc.tensor.matmul(up[:, :n], ws["wn1b"][:], aggsb[:, a:a + n],
                                     start=False, stop=True)
                    nc.vector.tensor_scalar(u1[:, a:a + n], up[:, :n],
                                            bs["bn1"][:], 0.0,
                                            op0=OP.add, op1=OP.max)
                u2 = np_.tile([P, n_loc_pad], F32, tag="u2")
                for a in range(0, n_loc_pad, 512):
                    n = min(512, n_loc_pad - a)
                    up2 = npp.tile([P, 512], F32, tag="up")
                    nc.tensor.matmul(up2[:, :n], ws["wn2"][:], u1[:, a:a + n],
                                     start=True, stop=True)
                    nc.vector.tensor_scalar(u2[:, a:a + n], up2[:, :n],
                                            bs["bn2"][:], None, op0=OP.add)
                # BN stats over the real nodes
                stats = np_.tile([P, 2], F32, tag="stats")
                nc.vector.tensor_reduce(stats[:, 0:1], u2[:, :N_REAL],
                                        axis=mybir.AxisListType.X, op=OP.add)
                sq = np_.tile([P, N_REAL], F32, tag="sq")
                nc.vector.tensor_tensor(sq[:], u2[:, :N_REAL], u2[:, :N_REAL],
                                        op=OP.mult)
                nc.vector.tensor_reduce(stats[:, 1:2], sq[:],
                                        axis=mybir.AxisListType.X, op=OP.add)
                tot = np_.tile([P, 2], F32, tag="tot")
                if USE_COLL:
                    cin = dp.tile([P, 2], F32, tag="cin")
                    cout = dp.tile([P, 2], F32, tag="cout")
                    nc.gpsimd.dma_start(cin[:], stats[:])
                    nc.gpsimd.collective_compute(
                        "AllReduce", OP.add, ins=[cin.opt()], outs=[cout.opt()],
                        replica_groups=[list(range(N_CORES))])
                    nc.gpsimd.dma_start(tot[:], cout[:])
                else:
                    # debug fallback: approximate global stats from local shard
                    nc.vector.tensor_scalar_mul(tot[:], stats[:], float(N_CORES))
                mean = np_.tile([P, 1], F32, tag="mean")
                nc.vector.tensor_scalar_mul(mean[:], tot[:, 0:1], 1.0 / s["N"])
                ex2 = np_.tile([P, 1], F32, tag="ex2")
                nc.vector.tensor_scalar_mul(ex2[:], tot[:, 1:2], 1.0 / s["N"])
                m2 = np_.tile([P, 1], F32, tag="m2")
                nc.vector.tensor_tensor(m2[:], mean[:], mean[:], op=OP.mult)
                var = np_.tile([P, 1], F32, tag="var")
                nc.vector.tensor_tensor(var[:], ex2[:], m2[:], op=OP.subtract)
                epst = np_.tile([P, 1], F32, tag="epst")
                nc.vector.memset(epst[:], BN_EPS)
                srt = np_.tile([P, 1], F32, tag="srt")
                nc.scalar.activation(srt[:], var[:], AF.Sqrt, bias=epst[:])
                rstd = np_.tile([P, 1], F32, tag="rstd")
                nc.vector.reciprocal(rstd[:], srt[:])
                scal = np_.tile([P, 1], F32, tag="scal")
                nc.vector.tensor_tensor(scal[:], rstd[:], bs["gam"][:], op=OP.mult)
                msc = np_.tile([P, 1], F32, tag="msc")
                nc.vector.tensor_tensor(msc[:], mean[:], scal[:], op=OP.mult)
                shif = np_.tile([P, 1], F32, tag="shif")
                nc.vector.tensor_tensor(shif[:], bs["bet"][:], msc[:], op=OP.subtract)
                un = np_.tile([P, n_loc_pad], F32, tag="un")
                nc.vector.tensor_scalar(un[:], u2[:], scal[:], shif[:],
                                        op0=OP.mult, op1=OP.add)
                ident = np_.tile([P, P], F32, tag="ident")
                make_identity(nc, ident[:])
                for t in range(n_loc_pad // P):
                    tp = tpp.tile([P, P], F32, tag="tp")
                    nc.tensor.transpose(tp[:], un[:, t * P:(t + 1) * P], ident[:])
                    nf32t = nt.tile([P, P], F32, tag="nf32t")
                    nc.sync.dma_start(nf32t[:], nf32_d[t * P:(t + 1) * P, :])
                    ot = nt.tile([P, P], F32, tag="ot")
                    nc.vector.tensor_tensor(ot[:], tp[:], nf32t[:], op=OP.add)
                    nc.sync.dma_start(out_d[t * P:(t + 1) * P, :], ot[:])
    nc.compile()
    return nc


def kernel(node_features, edge_features, We1, be1, We2, be2, Wn1, bn1, Wn2,
           bn2, gamma, beta, edge_index, _profile=None):
    sched, in_maps = _prep(np.asarray(node_features, np.float32),
                           np.asarray(edge_features, np.float32),
                           np.asarray(edge_index))
    shared = _shared_inputs(We1, be1, We2, be2, Wn1, bn1, Wn2, bn2, gamma,
                            beta, sched["H"])
    for m in in_maps:
        m.update(shared)
    nc = _build_program(sched)
    t0 = time.perf_counter()
    res = run_bass_kernel_spmd(nc, in_maps, core_ids=list(range(N_CORES)))
    spmd_ns = (time.perf_counter() - t0) * 1e9
    n_local, N = sched["n_local"], sched["N"]
    out = np.concatenate(
        [res.results[c]["out"][:n_local] for c in range(N_CORES)], axis=0)[:N]
    if _profile is not None:
        _profile["exec_time_ns"] = res.exec_time_ns
        _profile["spmd_wall_ns"] = spmd_ns
    return out.astype(np.float32)



# revision 17
# speedup vs baseline: 1.5187x; 1.5187x over previous
"""CrystalGraphConv Bass kernel for 8 TRN2 NeuronCores.

Strategy (edge-parallel, dst-sharded), v2 — optimized for end-to-end dispatch
time (the metric includes PJRT dispatch, NEFF load and host->device transfer):
  - Nodes partitioned into 8 contiguous ranges of 1250; edge e owned by the
    core owning dst[e], so segment_sum is core-local.
  - Per core, dst-space split into 10 windows of 128 nodes; edges grouped by
    window, padded to 128-edge tiles (counts maxed over cores so the SPMD
    program is identical everywhere).
  - Node table extended to [10001, 132] bf16: col 128 holds node_id % 128 (the
    one-hot offset within any dst window) and row 10000 is a zero pad row with
    offset 200, so pad edges one-hot to nothing. The dst gather thus carries
    both features and scatter offsets - no separate offset tensors.
  - Edge features shipped as fp8 e4m3 and upcast on-device.
  - Edge phase runs as 10 hardware For_i loops (one per window, ~33 iters):
    per 128-edge tile, two indirect gathers, TensorE transposes, the edge MLP,
    gated message, one-hot scatter matmul into a PSUM [128, 1280] accumulator.
    All loop-variant addressing is on DMA ops only.
  - Node MLP + BN node-sharded; BN statistics via a [128, 2] AllReduce.
"""

import sys, time

sys.path.insert(0, "/opt/trn_rl_repo")

import numpy as np
import ml_dtypes

import concourse.bacc as bacc
import concourse.bass as bass
import concourse.mybir as mybir
import concourse.tile as tile
from concourse.bass import ds
from concourse.bass_utils import run_bass_kernel_spmd
from concourse.masks import make_identity

BF16 = ml_dtypes.bfloat16
FP8 = ml_dtypes.float8_e4m3
N_CORES = 8
P = 128
WIN = 128
BN_EPS = 1e-5
PAD_OFF = 200.0  # offset col value for the pad row; >= WIN so one-hot is zero
F32 = mybir.dt.float32
BT = mybir.dt.bfloat16
F8 = mybir.dt.float8e4
I16 = mybir.dt.int16
I32 = mybir.dt.int32
AF = mybir.ActivationFunctionType
OP = mybir.AluOpType

DEBUG_DUMP = False  # adds intermediate DRAM outputs for sim debugging

N = 10000
H = 128
ED = 64
N_LOCAL = 1280          # node slots per core (multiple of 128; core 7 has pads)
N_WIN = 10              # dst windows per core
N_LOC_PAD = N_WIN * WIN  # 1280
TABW = 132              # extended table row width (128 feat + offmod + pad)
N_TAB = N + 1           # +1 zero pad row
PAD_NODES = N_CORES * N_LOC_PAD - N  # 240 pad node slots (all on core 7)


def _prep(node_features, edge_features, edge_index):
    """Host-side sharding/schedule. Returns (schedule, per-core input dicts)."""
    src = edge_index[0].astype(np.int64)
    dst = edge_index[1].astype(np.int64)

    core_of = dst // N_LOCAL
    loc = dst - core_of * N_LOCAL
    w_of = loc >> 7

    # stable sort edges by (core, window)
    order = np.argsort(core_of * N_WIN + w_of, kind="stable")
    counts = np.bincount((core_of * N_WIN + w_of)[order],
                         minlength=N_CORES * N_WIN).reshape(N_CORES, N_WIN)
    tiles_w = np.maximum(1, (counts.max(axis=0) + P - 1) // P).astype(np.int64)
    E_w = tiles_w * P
    O_w = np.concatenate([[0], np.cumsum(E_w)])
    E_CAP = int(O_w[-1])
    T_w = np.concatenate([[0], np.cumsum(tiles_w)])
    T_tot = int(T_w[-1])

    # extended node table (shared by all cores)
    nft = np.zeros((N_TAB, TABW), dtype=BF16)
    nft[:N, :H] = node_features.astype(BF16)
    nft[:N, H] = (np.arange(N) % WIN).astype(BF16)
    nft[N, H] = BF16(PAD_OFF)

    ef32 = edge_features
    csum = np.concatenate([[0], np.cumsum(counts.reshape(-1))])

    in_maps = []
    for c in range(N_CORES):
        g_src = np.full(E_CAP, N, dtype=np.int16)  # pad -> zero row
        g_dst = np.full(E_CAP, N, dtype=np.int16)  # pad -> zero row (off 200)
        ef8 = np.zeros((ED, E_CAP), dtype=FP8)
        for w in range(N_WIN):
            k = int(counts[c, w])
            ids = order[csum[c * N_WIN + w]:csum[c * N_WIN + w] + k]
            o = int(O_w[w])
            g_src[o:o + k] = src[ids]
            g_dst[o:o + k] = dst[ids]
            ef8[:, o:o + k] = ef32[ids].T.astype(FP8)
        sidx = g_src.reshape(-1, P).T.astype(np.int32)   # [128, T_tot]
        didx = g_dst.reshape(-1, P).T.astype(np.int32)
        base = c * N_LOCAL
        nid = base + (np.arange(N_WIN)[None, :] * P + np.arange(P)[:, None])
        nid = np.minimum(nid, N)  # pad slots -> zero pad row
        in_maps.append({
            "nft": nft,
            "ef8": ef8,
            "sidx": sidx,
            "didx": didx,
            "nidx": nid.astype(np.int32),
        })

    sched = dict(E_CAP=E_CAP, tiles_w=tiles_w.tolist(), O_w=O_w.tolist(),
                 T_w=T_w.tolist(), T_tot=T_tot)
    return sched, in_maps


def _shared_inputs(We1, be1, We2, be2, Wn1, bn1, Wn2, bn2, gamma, beta):
    wpk = np.zeros((P, 7 * H), dtype=BF16)
    wpk[:, 0 * H:1 * H] = np.asarray(We1[:H], BF16)
    wpk[:, 1 * H:2 * H] = np.asarray(We1[H:2 * H], BF16)
    wpk[:ED, 2 * H:3 * H] = np.asarray(We1[2 * H:], BF16)
    wpk[:, 3 * H:4 * H] = np.asarray(We2, BF16)
    wpk[:, 4 * H:5 * H] = np.asarray(Wn1[:H], BF16)
    wpk[:, 5 * H:6 * H] = np.asarray(Wn1[H:], BF16)
    wpk[:, 6 * H:7 * H] = np.asarray(Wn2, BF16)
    bpk = np.zeros((P, 8), dtype=np.float32)
    for j, v in enumerate([be1, be2, bn1, bn2, gamma, beta]):
        bpk[:, j] = np.asarray(v, np.float32)
    return {"wpk": wpk, "bpk": bpk}


def _build_program(s):
    E_CAP, T_tot = s["E_CAP"], s["T_tot"]
    tiles_w, O_w, T_w = s["tiles_w"], s["O_w"], s["T_w"]

    nc = bacc.Bacc("TRN2", target_bir_lowering=False, debug=False,
                   num_devices=N_CORES)
    dt = lambda n, sh, d, k: nc.dram_tensor(n, sh, d, kind=k).ap()
    IN = "ExternalInput"
    nft_d = dt("nft", [N_TAB, TABW], BT, IN)
    ef8_d = dt("ef8", [ED, E_CAP], F8, IN)
    sidx_d = dt("sidx", [P, T_tot], I32, IN)
    didx_d = dt("didx", [P, T_tot], I32, IN)
    nidx_d = dt("nidx", [P, N_WIN], I32, IN)
    wpk_d = dt("wpk", [P, 7 * H], BT, IN)
    bpk_d = dt("bpk", [P, 8], F32, IN)
    out_d = dt("out", [N_LOC_PAD, H], F32, "ExternalOutput")
    if DEBUG_DUMP:
        dbg_agg_d = dt("dbg_agg", [P, N_LOC_PAD], F32, "ExternalOutput")
        dbg_nft_d = dt("dbg_nft", [P, N_LOC_PAD], F32, "ExternalOutput")
        dbg_u2_d = dt("dbg_u2", [P, N_LOC_PAD], F32, "ExternalOutput")
        dbg_st_d = dt("dbg_st", [P, 4], F32, "ExternalOutput")

    with tile.TileContext(nc) as tc:
        with tc.tile_pool(name="const", bufs=1) as cp, \
             tc.tile_pool(name="aggps", bufs=1, space="PSUM") as aggpool:
            # ---- persistent constants to SBUF ----
            wpk = cp.tile([P, 7 * H], BT, tag="wpk")
            nc.sync.dma_start(wpk[:], wpk_d[:])
            w_src = wpk[:, 0 * H:1 * H]
            w_dst = wpk[:, 1 * H:2 * H]
            w_ef = wpk[:ED, 2 * H:3 * H]
            we2 = wpk[:, 3 * H:4 * H]
            wn1a = wpk[:, 4 * H:5 * H]
            wn1b = wpk[:, 5 * H:6 * H]
            wn2 = wpk[:, 6 * H:7 * H]
            bpk = cp.tile([P, 8], F32, tag="bpk")
            nc.sync.dma_start(bpk[:], bpk_d[:])
            be1, be2, bn1, bn2, gam, bet = (bpk[:, j:j + 1] for j in range(6))
            identE = cp.tile([P, P], BT, tag="identE")
            make_identity(nc, identE[:])
            identF = cp.tile([P, P], F32, tag="identF")
            make_identity(nc, identF[:])
            iota16 = cp.tile([P, WIN], I16, tag="iota16")
            nc.gpsimd.iota(iota16[:], pattern=[[1, WIN]], base=0,
                           channel_multiplier=0)
            iota = cp.tile([P, WIN], BT, tag="iota")
            nc.vector.tensor_copy(iota[:], iota16[:])
            zlhs = cp.tile([P, P], BT, tag="zlhs")
            nc.vector.memset(zlhs[:], 0.0)
            zrhs = cp.tile([P, 512], BT, tag="zrhs")
            nc.vector.memset(zrhs[:], 0.0)

            agg = aggpool.tile([P, N_LOC_PAD], F32, tag="agg")
            # zero-init agg; groups stay open so loop scatters can accumulate
            for a in range(0, N_LOC_PAD, 512):
                n = min(512, N_LOC_PAD - a)
                nc.tensor.matmul(agg[:, a:a + n], zlhs[:], zrhs[:, :n],
                                 start=True, stop=False)

            # ---- edge phase: one hardware loop per dst window ----
            with tc.tile_pool(name="gath", bufs=2) as gp, \
                 tc.tile_pool(name="work", bufs=2) as wp, \
                 tc.tile_pool(name="tps", bufs=2, space="PSUM") as tpp, \
                 tc.tile_pool(name="mps", bufs=2, space="PSUM") as mpp:
                for w in range(N_WIN):
                    tw, o, t0 = tiles_w[w], O_w[w], T_w[w]
                    with tc.For_i(0, tw, 1) as t:
                        # indirect-DMA offset APs must be physical, so stage
                        # the loop-variant index columns into fixed tiles
                        scol = gp.tile([P, 1], I32, tag="scol")
                        nc.sync.dma_start(scol[:], sidx_d[:, ds(t0 + t, 1)])
                        dcol = gp.tile([P, 1], I32, tag="dcol")
                        nc.sync.dma_start(dcol[:], didx_d[:, ds(t0 + t, 1)])
                        sEM = gp.tile([P, TABW], BT, tag="sEM")
                        nc.gpsimd.indirect_dma_start(
                            sEM[:], None, nft_d[:],
                            bass.IndirectOffsetOnAxis(ap=scol[:], axis=0))
                        dEM = gp.tile([P, TABW], BT, tag="dEM")
                        nc.gpsimd.indirect_dma_start(
                            dEM[:], None, nft_d[:],
                            bass.IndirectOffsetOnAxis(ap=dcol[:], axis=0))
                        ef8t = gp.tile([ED, P], F8, tag="ef8t")
                        nc.sync.dma_start(ef8t[:],
                                          ef8_d[:, ds(o + t * P, P)])
                        efb = gp.tile([ED, P], BT, tag="efb")
                        nc.vector.tensor_copy(efb[:], ef8t[:])
                        tp1 = tpp.tile([P, P], BT, tag="tp")
                        nc.tensor.transpose(tp1[:], sEM[:, :H], identE[:])
                        srcT = wp.tile([P, P], BT, tag="srcT")
                        nc.vector.tensor_copy(srcT[:], tp1[:])
                        tp2 = tpp.tile([P, P], BT, tag="tp")
                        nc.tensor.transpose(tp2[:], dEM[:, :H], identE[:])
                        dstT = wp.tile([P, P], BT, tag="dstT")
                        nc.vector.tensor_copy(dstT[:], tp2[:])
                        hps = mpp.tile([P, P], F32, tag="mp")
                        nc.tensor.matmul(hps[:], w_src, srcT[:],
                                         start=True, stop=False)
                        nc.tensor.matmul(hps[:], w_dst, dstT[:],
                                         start=False, stop=False)
                        nc.tensor.matmul(hps[:], w_ef, efb[:],
                                         start=False, stop=True)
                        hsb = wp.tile([P, P], BT, tag="hsb")
                        nc.vector.tensor_scalar(hsb[:], hps[:], be1, 0.0,
                                                op0=OP.add, op1=OP.max)
                        gps = mpp.tile([P, P], F32, tag="mp")
                        nc.tensor.matmul(gps[:], we2, hsb[:],
                                         start=True, stop=True)
                        gT = wp.tile([P, P], BT, tag="gT")
                        nc.scalar.activation(gT[:], gps[:], AF.Sigmoid,
                                             bias=be2)
                        msgF = wp.tile([P, P], BT, tag="msgF")
                        nc.vector.tensor_tensor(msgF[:], srcT[:], gT[:],
                                                op=OP.mult)
                        tp3 = tpp.tile([P, P], BT, tag="tp")
                        nc.tensor.transpose(tp3[:], msgF[:], identE[:])
                        msgE = wp.tile([P, P], BT, tag="msgE")
                        nc.vector.tensor_copy(msgE[:], tp3[:])
                        hot = wp.tile([P, WIN], BT, tag="hot")
                        nc.vector.tensor_tensor(
                            hot[:], dEM[:, H:H + 1].to_broadcast([P, WIN]),
                            iota[:], op=OP.is_equal)
                        nc.tensor.matmul(agg[:, w * WIN:(w + 1) * WIN],
                                         msgE[:], hot[:],
                                         start=False, stop=False)
                # close the accumulation groups (adds zero; stop is
                # scheduler/sim metadata, a no-op on hardware)
                for a in range(0, N_LOC_PAD, 512):
                    n = min(512, N_LOC_PAD - a)
                    nc.tensor.matmul(agg[:, a:a + n], zlhs[:], zrhs[:, :n],
                                     start=False, stop=True)

            # ---- node phase ----
            with tc.tile_pool(name="node", bufs=1) as np_, \
                 tc.tile_pool(name="nps", bufs=1, space="PSUM") as npp, \
                 tc.tile_pool(name="ntmp", bufs=2) as nt, \
                 tc.tile_pool(name="dram", bufs=1, space="DRAM") as dp:
                nidx = np_.tile([P, N_WIN], I32, tag="nidx")
                nc.sync.dma_start(nidx[:], nidx_d[:])
                nfN = []
                for j in range(N_WIN):
                    nfj = np_.tile([P, TABW], BT, tag=f"nfN{j}")
                    nc.gpsimd.indirect_dma_start(
                        nfj[:], None, nft_d[:],
                        bass.IndirectOffsetOnAxis(ap=nidx[:, j:j + 1], axis=0))
                    nfN.append(nfj)
                nfT = np_.tile([P, N_LOC_PAD], BT, tag="nfT")
                for j in range(N_WIN):
                    tpj = npp.tile([P, P], BT, tag="np")
                    nc.tensor.transpose(tpj[:], nfN[j][:, :H], identE[:])
                    nc.vector.tensor_copy(nfT[:, j * P:(j + 1) * P], tpj[:])
                aggsb = np_.tile([P, N_LOC_PAD], BT, tag="aggsb")
                nc.vector.tensor_copy(aggsb[:], agg[:])
                if DEBUG_DUMP:
                    dba = np_.tile([P, N_LOC_PAD], F32, tag="dba")
                    nc.vector.tensor_copy(dba[:], agg[:])
                    nc.sync.dma_start(dbg_agg_d[:], dba[:])
                    dbn = np_.tile([P, N_LOC_PAD], F32, tag="dbn")
                    nc.vector.tensor_copy(dbn[:], nfT[:])
                    nc.sync.dma_start(dbg_nft_d[:], dbn[:])
                u1 = np_.tile([P, N_LOC_PAD], BT, tag="u1")
                for a in range(0, N_LOC_PAD, 512):
                    n = min(512, N_LOC_PAD - a)
                    up = npp.tile([P, 512], F32, tag="np5")
                    nc.tensor.matmul(up[:, :n], wn1a, nfT[:, a:a + n],
                                     start=True, stop=False)
                    nc.tensor.matmul(up[:, :n], wn1b, aggsb[:, a:a + n],
                                     start=False, stop=True)
                    nc.vector.tensor_scalar(u1[:, a:a + n], up[:, :n],
                                            bn1, 0.0, op0=OP.add, op1=OP.max)
                u2 = np_.tile([P, N_LOC_PAD], F32, tag="u2")
                for a in range(0, N_LOC_PAD, 512):
                    n = min(512, N_LOC_PAD - a)
                    up2 = npp.tile([P, 512], F32, tag="np5")
                    nc.tensor.matmul(up2[:, :n], wn2, u1[:, a:a + n],
                                     start=True, stop=True)
                    nc.vector.tensor_scalar(u2[:, a:a + n], up2[:, :n],
                                            bn2, None, op0=OP.add)
                # BN stats over the real nodes
                stats = np_.tile([P, 2], F32, tag="stats")
                nc.vector.tensor_reduce(stats[:, 0:1], u2[:, :N_LOCAL],
                                        axis=mybir.AxisListType.X, op=OP.add)
                sq = np_.tile([P, N_LOCAL], F32, tag="sq")
                nc.vector.tensor_tensor(sq[:], u2[:, :N_LOCAL],
                                        u2[:, :N_LOCAL], op=OP.mult)
                nc.vector.tensor_reduce(stats[:, 1:2], sq[:],
                                        axis=mybir.AxisListType.X, op=OP.add)
                tot = np_.tile([P, 2], F32, tag="tot")
                cin = dp.tile([P, 2], F32, tag="cin")
                cout = dp.tile([P, 2], F32, tag="cout")
                nc.gpsimd.dma_start(cin[:], stats[:])
                nc.gpsimd.collective_compute(
                    "AllReduce", OP.add, ins=[cin.opt()], outs=[cout.opt()],
                    replica_groups=[list(range(N_CORES))])
                nc.gpsimd.dma_start(tot[:], cout[:])
                # pad-node correction: subtract PAD_NODES * [u2p, u2p^2]
                # where u2p = Wn2^T relu(bn1) + bn2, bit-identical to the
                # real pad-column compute (zero features, zero aggregate).
                z1 = np_.tile([P, 1], F32, tag="z1")
                nc.vector.memset(z1[:], 0.0)
                u1p = np_.tile([P, 1], BT, tag="u1p")
                nc.vector.tensor_scalar(u1p[:], z1[:], bn1, 0.0,
                                        op0=OP.add, op1=OP.max)
                u2pp = npp.tile([P, 1], F32, tag="np1")
                nc.tensor.matmul(u2pp[:], wn2, u1p[:], start=True, stop=True)
                u2p = np_.tile([P, 1], F32, tag="u2p")
                nc.vector.tensor_scalar(u2p[:], u2pp[:], bn2, None,
                                        op0=OP.add)
                u2p2 = np_.tile([P, 1], F32, tag="u2p2")
                nc.vector.tensor_tensor(u2p2[:], u2p[:], u2p[:], op=OP.mult)
                c1 = np_.tile([P, 1], F32, tag="c1")
                nc.vector.tensor_scalar_mul(c1[:], u2p[:], float(PAD_NODES))
                c2 = np_.tile([P, 1], F32, tag="c2")
                nc.vector.tensor_scalar_mul(c2[:], u2p2[:], float(PAD_NODES))
                t0c = np_.tile([P, 1], F32, tag="t0c")
                nc.vector.tensor_tensor(t0c[:], tot[:, 0:1], c1[:],
                                        op=OP.subtract)
                t1c = np_.tile([P, 1], F32, tag="t1c")
                nc.vector.tensor_tensor(t1c[:], tot[:, 1:2], c2[:],
                                        op=OP.subtract)
                mean = np_.tile([P, 1], F32, tag="mean")
                nc.vector.tensor_scalar_mul(mean[:], t0c[:], 1.0 / N)
                ex2 = np_.tile([P, 1], F32, tag="ex2")
                nc.vector.tensor_scalar_mul(ex2[:], t1c[:], 1.0 / N)
                m2 = np_.tile([P, 1], F32, tag="m2")
                nc.vector.tensor_tensor(m2[:], mean[:], mean[:], op=OP.mult)
                var = np_.tile([P, 1], F32, tag="var")
                nc.vector.tensor_tensor(var[:], ex2[:], m2[:], op=OP.subtract)
                epst = np_.tile([P, 1], F32, tag="epst")
                nc.vector.memset(epst[:], BN_EPS)
                srt = np_.tile([P, 1], F32, tag="srt")
                nc.scalar.activation(srt[:], var[:], AF.Sqrt, bias=epst[:])
                rstd = np_.tile([P, 1], F32, tag="rstd")
                nc.vector.reciprocal(rstd[:], srt[:])
                scal = np_.tile([P, 1], F32, tag="scal")
                nc.vector.tensor_tensor(scal[:], rstd[:], gam, op=OP.mult)
                msc = np_.tile([P, 1], F32, tag="msc")
                nc.vector.tensor_tensor(msc[:], mean[:], scal[:], op=OP.mult)
                shif = np_.tile([P, 1], F32, tag="shif")
                nc.vector.tensor_tensor(shif[:], bet, msc[:], op=OP.subtract)
                if DEBUG_DUMP:
                    nc.sync.dma_start(dbg_u2_d[:], u2[:])
                    dst_ = np_.tile([P, 4], F32, tag="dst_")
                    nc.vector.tensor_copy(dst_[:, 0:1], mean[:])
                    nc.vector.tensor_copy(dst_[:, 1:2], var[:])
                    nc.vector.tensor_copy(dst_[:, 2:3], scal[:])
                    nc.vector.tensor_copy(dst_[:, 3:4], shif[:])
                    nc.sync.dma_start(dbg_st_d[:], dst_[:])
                un = np_.tile([P, N_LOC_PAD], F32, tag="un")
                nc.vector.tensor_scalar(un[:], u2[:], scal[:], shif[:],
                                        op0=OP.mult, op1=OP.add)
                for t in range(N_WIN):
                    tp = npp.tile([P, P], F32, tag="np")
                    nc.tensor.transpose(tp[:], un[:, t * P:(t + 1) * P],
                                        identF[:])
                    nfr = nt.tile([P, P], F32, tag="nfr")
                    nc.vector.tensor_copy(nfr[:], nfN[t][:, :H])
                    ot = nt.tile([P, P], F32, tag="ot")
                    nc.vector.tensor_tensor(ot[:], tp[:], nfr[:], op=OP.add)
                    nc.sync.dma_start(out_d[t * P:(t + 1) * P, :], ot[:])
    nc.compile()
    return nc


def kernel(node_features, edge_features, We1, be1, We2, be2, Wn1, bn1, Wn2,
           bn2, gamma, beta, edge_index, _profile=None):
    sched, in_maps = _prep(np.asarray(node_features, np.float32),
                           np.asarray(edge_features, np.float32),
                           np.asarray(edge_index))
    shared = _shared_inputs(We1, be1, We2, be2, Wn1, bn1, Wn2, bn2, gamma,
                            beta)
    for m in in_maps:
        m.update(shared)
    nc = _build_program(sched)
    t0 = time.perf_counter()
    res = run_bass_kernel_spmd(nc, in_maps, core_ids=list(range(N_CORES)))
    spmd_ns = (time.perf_counter() - t0) * 1e9
    out = np.concatenate(
        [res.results[c]["out"][:min(N_LOCAL, N - c * N_LOCAL)]
         for c in range(N_CORES)], axis=0)
    if _profile is not None:
        _profile["exec_time_ns"] = res.exec_time_ns
        _profile["spmd_wall_ns"] = spmd_ns
    return out.astype(np.float32)
